# revision 1
# baseline (speedup 1.0000x reference)
"""CoLoRConv2d on 8 TRN2 NeuronCores.

Math: the reference builds per-sample modulated conv weights
    w[b] = CONV_SCALE * (weight + lora_delta) * s[b, ic] * demod[b, oc]
and runs a batch-as-groups 3x3 conv. Because the modulation factors
separate per-channel, the conv weights are batch-invariant:
    out[b, oc] = d[b, oc] * conv2d(s[b, ic] * x[b, ic], w5)
with w5 = weight + lora_delta shared across the batch, s the style
modulation and d = CONV_SCALE * demod computed analytically on host
(demod^2 sums separate as sum_i (sum_k w5^2) * s^2).

Sharding: data-parallel over batch - core b handles sample b. Each core
prescales + zero-pads its input into SBUF [128, 66, 66] tiles per
input-channel block, then runs 1152 accumulating fp32r matmuls
(K=128 ic x N=512 pixels, 9 taps x 4 ic-blocks accumulated in PSUM)
and applies the per-output-channel demod scale on PSUM evacuation.
"""
import math
import os

import numpy as np

import concourse.bass as bass
import concourse.mybir as mybir
import concourse.tile as tile
from concourse import bacc
from concourse.bass_utils import run_bass_kernel_spmd

IN_CH = 512
OUT_CH = 512
K = 3
STYLE_DIM = 512
RANK = 4
CONV_SCALE = 1.0 / math.sqrt(IN_CH * K * K)
MOD_SCALE = 1.0 / math.sqrt(STYLE_DIM)
EPS = 1e-8
B, H, W = 8, 64, 64
HW = H * W
P = 128
NBLK = IN_CH // P            # 4 input-channel blocks
OBLK = OUT_CH // P           # 4 output-channel blocks
HP = H + 2                   # padded rows
WP = W + 2                   # padded cols
ROWS_PER_MM = 8              # 8 rows x 64 cols = 512 = one PSUM bank
QCHUNKS = H // ROWS_PER_MM   # 8 row-chunks per plane

F32 = mybir.dt.float32
F32R = mybir.dt.float32r
COPY = mybir.ActivationFunctionType.Copy

_CACHED_NC = None
LAST_RESULT = None


def _build(loop_n=None, loop_scope="all"):
    """Build the per-core program. loop_n wraps the body (loop_scope="all")
    or just the conv+evac+store section (loop_scope="conv") in an in-NEFF
    hardware loop (benchmarking only; kernel() uses loop_n=None)."""
    import contextlib

    nc = bacc.Bacc(None)
    x = nc.declare_dram_parameter("x", [IN_CH, HW], F32, isOutput=False)
    wt = nc.declare_dram_parameter("wt", [NBLK, P, K * K, OUT_CH], F32R, isOutput=False)
    sv = nc.declare_dram_parameter("sv", [P, NBLK], F32, isOutput=False)
    dv = nc.declare_dram_parameter("dv", [P, OBLK], F32, isOutput=False)
    out = nc.declare_dram_parameter("out", [OUT_CH, HW], F32, isOutput=True)

    with tile.TileContext(nc) as tc:
        loop_cm = (tc.For_i(0, loop_n, 1) if loop_n and loop_scope == "all"
                   else contextlib.nullcontext())
        with (
            loop_cm,
            tc.tile_pool(name="singles", bufs=1) as singles,
            tc.tile_pool(name="wpool", bufs=1) as wpool,
            tc.tile_pool(name="xpad", bufs=1) as xpad_pool,
            tc.tile_pool(name="raw", bufs=3) as raw_pool,
            tc.tile_pool(name="stage", bufs=6) as stage_pool,
            tc.tile_pool(name="psum", bufs=8, space="PSUM") as psum_pool,
        ):
            svt = singles.tile([P, NBLK], F32, tag="svt")
            dvt = singles.tile([P, OBLK], F32, tag="dvt")
            nc.sync.dma_start(out=svt[:], in_=sv[:])
            nc.sync.dma_start(out=dvt[:], in_=dv[:])

            # Weights: [ic_in_blk, tap, oc] per ic block, fp32r bits straight
            # from DRAM (PE rounds on read; dtype must be f32r end-to-end to
            # satisfy the BIR verifier).
            w_sb = []
            for blk in range(NBLK):
                t = wpool.tile([P, K * K, OUT_CH], F32R, tag=f"w{blk}")
                nc.sync.dma_start(out=t[:], in_=wt[blk])
                w_sb.append(t)

            # Padded, style-prescaled input planes, rounded to f32r by the
            # scalar-engine copy that writes them.
            xpad = []
            for blk in range(NBLK):
                t = xpad_pool.tile([P, HP, WP], F32R, tag=f"xp{blk}")
                # zero borders: top row, bottom row, left col, right col
                # (memset lacks f32r support; zero bits are dtype-agnostic)
                nc.vector.memset(t[:, 0, :].bitcast(F32), 0.0)
                nc.vector.memset(t[:, HP - 1, :].bitcast(F32), 0.0)
                nc.vector.memset(t[:, 1 : HP - 1, 0].bitcast(F32), 0.0)
                nc.vector.memset(t[:, 1 : HP - 1, WP - 1].bitcast(F32), 0.0)
                xpad.append(t)

            # interior = s[ic] * x, in row-chunks so DMA/compute pipeline
            CH_ROWS = 16
            for blk in range(NBLK):
                for c in range(H // CH_ROWS):
                    raw = raw_pool.tile([P, CH_ROWS, W], F32, tag="raw")
                    nc.sync.dma_start(
                        out=raw[:],
                        in_=x[blk * P : (blk + 1) * P,
                             c * CH_ROWS * W : (c + 1) * CH_ROWS * W]
                        .rearrange("p (r w) -> p r w", r=CH_ROWS),
                    )
                    nc.scalar.activation(
                        out=xpad[blk][:, 1 + c * CH_ROWS : 1 + (c + 1) * CH_ROWS, 1 : 1 + W],
                        in_=raw[:],
                        func=COPY,
                        scale=svt[:, blk : blk + 1],
                    )

            # Conv: for each (oc block, row chunk): accumulate 4 ic-blocks x
            # 9 taps into one PSUM bank, demod-scale on evacuation, DMA out.
            conv_cm = (tc.For_i(0, loop_n, 1) if loop_n and loop_scope == "conv"
                       else contextlib.nullcontext())
            conv_cm.__enter__()
            for ocb in range(OBLK):
                for q in range(QCHUNKS):
                    r0 = q * ROWS_PER_MM
                    ps = psum_pool.tile([P, ROWS_PER_MM, W], F32, tag="ps")
                    first = True
                    for blk in range(NBLK):
                        for tap in range(K * K):
                            dy, dx = tap // K, tap % K
                            nc.tensor.matmul(
                                ps[:],
                                w_sb[blk][:, tap, ocb * P : (ocb + 1) * P],
                                xpad[blk][:, r0 + dy : r0 + dy + ROWS_PER_MM,
                                          dx : dx + W],
                                start=first,
                                stop=(blk == NBLK - 1 and tap == K * K - 1),
                            )
                            first = False
                    st = stage_pool.tile([P, ROWS_PER_MM * W], F32, tag="st")
                    nc.scalar.activation(
                        out=st[:],
                        in_=ps[:].rearrange("p r w -> p (r w)"),
                        func=COPY,
                        scale=dvt[:, ocb : ocb + 1],
                    )
                    nc.sync.dma_start(
                        out=out[ocb * P : (ocb + 1) * P,
                                r0 * W : (r0 + ROWS_PER_MM) * W],
                        in_=st[:],
                    )
            conv_cm.__exit__(None, None, None)
    nc.finalize()
    return nc


# ---------------------------------------------------------------------------
# Winograd F(2,3) applied along the row (dy) axis only: 4 transformed
# components replace the 3 vertical taps, 2 output rows per tile ->
# 2/3 of the matmul work of the direct conv. Columns stay direct (the
# dx shifts reuse the 66-wide zero-padded layout). The chip is power-
# limited on dense fp32r matmul with all 8 cores active, so wall time
# tracks total multiply count - fewer MACs, proportionally faster.
WINO_BT = [  # V[c] = sum_r BT[c][r] * d[r]   (d = 4 padded input rows)
    [(1.0, 0), (-1.0, 2)],
    [(1.0, 1), (1.0, 2)],
    [(-1.0, 1), (1.0, 2)],
    [(1.0, 1), (-1.0, 3)],
]
WINO_G = np.array([[1.0, 0.0, 0.0],
                   [0.5, 0.5, 0.5],
                   [0.5, -0.5, 0.5],
                   [0.0, 0.0, 1.0]])
WINO_AT = [  # out[a] = sum_c AT[a][c] * M[c]
    [(1.0, 0), (1.0, 1), (1.0, 2)],
    [(1.0, 1), (-1.0, 2), (-1.0, 3)],
]
WC = 4          # components
WM = 2          # output rows per tile
GTY = 8         # tile-rows per group
NGRP = H // (WM * GTY)   # 4 groups
NMM = GTY * W   # matmul free size = 512


def _emit_lincomb(nc, out, terms, tmp):
    """out = sum(coef * ap) on the vector engine.

    Chains through `tmp` (f32, SBUF); only the final op writes `out`, so
    an f32r `out` is produced by a rounding instruction (the BIR
    verifier requires fp32r matmul inputs to be rounded by their
    producer). The hardware allows at most one PSUM operand per
    instruction, so a two-tensor op is only emitted when at most one of
    the pair lives in PSUM."""
    from concourse.bass import MemorySpace

    mult = mybir.AluOpType.mult
    add = mybir.AluOpType.add
    sub = mybir.AluOpType.subtract
    terms = sorted(terms, key=lambda t: abs(abs(t[0]) - 1.0))  # +-1 first
    if len(terms) == 1:
        c0, a0 = terms[0]
        nc.vector.tensor_scalar_mul(out, a0, float(c0))
        return
    if not isinstance(tmp, (list, tuple)):
        tmp = [tmp, tmp]
    (c0, a0), (c1, a1) = terms[0], terms[1]
    can_pair = not (a0.space == MemorySpace.PSUM and a1.space == MemorySpace.PSUM)
    dest = out if len(terms) == 2 else tmp[0]
    if can_pair and c0 == 1.0:
        nc.vector.scalar_tensor_tensor(dest, a1, float(c1), a0, mult, add)
    elif can_pair and c0 == -1.0:
        nc.vector.scalar_tensor_tensor(dest, a1, float(c1), a0, mult, sub)
    else:
        nc.vector.tensor_scalar_mul(tmp[1], a0, float(c0))
        nc.vector.scalar_tensor_tensor(dest, a1, float(c1), tmp[1], mult, add)
    cur = 0
    for ci, ai in terms[2:-1]:
        nc.vector.scalar_tensor_tensor(tmp[1 - cur], ai, float(ci), tmp[cur], mult, add)
        cur = 1 - cur
    if len(terms) > 2:
        cl, al = terms[-1]
        nc.vector.scalar_tensor_tensor(out, al, float(cl), tmp[cur], mult, add)


def _build_wino(loop_n=None):
    import contextlib

    nc = bacc.Bacc(None)
    x = nc.declare_dram_parameter("x", [IN_CH, HW], F32, isOutput=False)
    wt = nc.declare_dram_parameter("wt", [NBLK, P, K, WC, OUT_CH], F32R, isOutput=False)
    sv = nc.declare_dram_parameter("sv", [P, NBLK], F32, isOutput=False)
    dv = nc.declare_dram_parameter("dv", [P, OBLK], F32, isOutput=False)
    out = nc.declare_dram_parameter("out", [OUT_CH, HW], F32, isOutput=True)

    SLAB_R = WM * (GTY - 1) + WC      # 18 padded rows per group slab

    with tile.TileContext(nc) as tc:
        loop_cm = tc.For_i(0, loop_n, 1) if loop_n else contextlib.nullcontext()
        with (
            loop_cm,
            tc.tile_pool(name="singles", bufs=1) as singles,
            tc.tile_pool(name="wpool", bufs=1) as wpool,
            tc.tile_pool(name="vpool", bufs=2) as vpool,
            tc.tile_pool(name="slab", bufs=2) as slabpool,
            tc.tile_pool(name="tmps", bufs=1) as tmppool,
            tc.tile_pool(name="raw", bufs=2) as rawpool,
            tc.tile_pool(name="acc", bufs=2) as accpool,
            tc.tile_pool(name="stage", bufs=2) as stagepool,
            tc.tile_pool(name="psum", bufs=2, space="PSUM") as psum_pool,
        ):
            svt = singles.tile([P, NBLK], F32, tag="svt")
            dvt = singles.tile([P, OBLK], F32, tag="dvt")
            nc.sync.dma_start(out=svt[:], in_=sv[:])
            nc.sync.dma_start(out=dvt[:], in_=dv[:])

            w_sb = []
            for blk in range(NBLK):
                t = wpool.tile([P, K, WC, OUT_CH], F32R, tag=f"w{blk}")
                nc.sync.dma_start(out=t[:], in_=wt[blk])
                w_sb.append(t)

            for g in range(NGRP):
                prow0 = g * WM * GTY          # first padded row of the slab
                xlo = max(prow0 - 1, 0)       # first input row
                xhi = min(prow0 + SLAB_R - 2, H - 1)  # last input row
                nrows = xhi - xlo + 1
                slab_lo = (xlo + 1) - prow0   # slab row of first input row

                vts = []
                for blk in range(NBLK):
                    raw = rawpool.tile([P, SLAB_R, W], F32, tag="raw")
                    nc.sync.dma_start(
                        out=raw[:, 0:nrows, :],
                        in_=x[blk * P : (blk + 1) * P, xlo * W : (xhi + 1) * W]
                        .rearrange("p (r w) -> p r w", r=nrows),
                    )
                    slab = slabpool.tile([P, SLAB_R, WP], F32, tag="slab")
                    nc.vector.memset(slab[:], 0.0)
                    nc.scalar.activation(
                        out=slab[:, slab_lo : slab_lo + nrows, 1 : 1 + W],
                        in_=raw[:, 0:nrows, :],
                        func=COPY,
                        scale=svt[:, blk : blk + 1],
                    )
                    vt = vpool.tile([P, WC, GTY, WP], F32R, tag=f"v{blk}")
                    tmp = [tmppool.tile([P, GTY, WP], F32, tag="tmp", name="tmp")[:],
                           tmppool.tile([P, GTY, WP], F32, tag="tmp2", name="tmp2")[:]]
                    for c in range(WC):
                        terms = [
                            (coef, slab[:, r : r + WM * (GTY - 1) + 1 : WM, :])
                            for coef, r in WINO_BT[c]
                        ]
                        _emit_lincomb(nc, vt[:, c], terms, tmp)
                    vts.append(vt)

                for ocb in range(OBLK):
                    ps = psum_pool.tile([P, WC, NMM], F32, tag="ps")
                    for blk in range(NBLK):
                        for dx in range(K):
                            for c in range(WC):
                                nc.tensor.matmul(
                                    ps[:, c].rearrange("p (t x) -> p t x", t=GTY),
                                    w_sb[blk][:, dx, c, ocb * P : (ocb + 1) * P],
                                    vts[blk][:, c, :, dx : dx + W],
                                    start=(blk == 0 and dx == 0),
                                    stop=(blk == NBLK - 1 and dx == K - 1),
                                )
                    acc = accpool.tile([P, WM, NMM], F32, tag="acc")
                    atmp = [tmppool.tile([P, NMM], F32, tag="atmp", name="atmp")[:],
                            tmppool.tile([P, NMM], F32, tag="atmp2", name="atmp2")[:]]
                    for a in range(WM):
                        terms = [(coef, ps[:, c]) for coef, c in WINO_AT[a]]
                        _emit_lincomb(nc, acc[:, a], terms, atmp)
                    st = stagepool.tile([P, WM * GTY, W], F32, tag="st")
                    for a in range(WM):
                        nc.scalar.activation(
                            out=st[:, a : WM * GTY : WM, :],
                            in_=acc[:, a].rearrange("p (t x) -> p t x", t=GTY),
                            func=COPY,
                            scale=dvt[:, ocb : ocb + 1],
                        )
                    nc.sync.dma_start(
                        out=out[ocb * P : (ocb + 1) * P,
                                g * WM * GTY * W : (g + 1) * WM * GTY * W],
                        in_=st[:],
                    )
    nc.finalize()
    return nc


# --- F(4,3): 6 components per 4 output rows = half the direct-conv MACs.
W4_BT = [
    [(4.0, 0), (-5.0, 2), (1.0, 4)],
    [(-4.0, 1), (-4.0, 2), (1.0, 3), (1.0, 4)],
    [(4.0, 1), (-4.0, 2), (-1.0, 3), (1.0, 4)],
    [(-2.0, 1), (-1.0, 2), (2.0, 3), (1.0, 4)],
    [(2.0, 1), (-1.0, 2), (-2.0, 3), (1.0, 4)],
    [(4.0, 1), (-5.0, 3), (1.0, 5)],
]
W4_G = np.array([[0.25, 0.0, 0.0],
                 [-1/6, -1/6, -1/6],
                 [-1/6, 1/6, -1/6],
                 [1/24, 1/12, 1/6],
                 [1/24, -1/12, 1/6],
                 [0.0, 0.0, 1.0]])
W4_AT = [
    [(1.0, 0), (1.0, 1), (1.0, 2), (1.0, 3), (1.0, 4)],
    [(1.0, 1), (-1.0, 2), (2.0, 3), (-2.0, 4)],
    [(1.0, 1), (1.0, 2), (4.0, 3), (4.0, 4)],
    [(1.0, 1), (-1.0, 2), (8.0, 3), (-8.0, 4), (1.0, 5)],
]
W4C = 6   # components
W4M = 4   # output rows per tile
G4TY = 4  # tile-rows per group -> N = 256
N4MM = G4TY * W


def _build_wino4(loop_n=None):
    """F(4,3) row-Winograd. Style scale is folded into the per-core
    weights on host, so x goes raw into the padded slab (direct DMA).
    The transformed weights (18 planes, 144 KB/partition) don't fit
    next to double-buffered V, so W streams per (group, oc-block)."""
    import contextlib

    nc = bacc.Bacc(None)
    x = nc.declare_dram_parameter("x", [IN_CH, HW], F32, isOutput=False)
    wt = nc.declare_dram_parameter("wt", [NBLK, P, K, W4C, OUT_CH], F32R, isOutput=False)
    dv = nc.declare_dram_parameter("dv", [P, OBLK], F32, isOutput=False)
    out = nc.declare_dram_parameter("out", [OUT_CH, HW], F32, isOutput=True)

    SLAB_R = W4M * (G4TY - 1) + W4C   # 18 padded rows per group slab
    NGRP4 = H // (W4M * G4TY)         # 4 groups

    with tile.TileContext(nc) as tc:
        loop_cm = tc.For_i(0, loop_n, 1) if loop_n else contextlib.nullcontext()
        with (
            loop_cm,
            tc.tile_pool(name="singles", bufs=1) as singles,
            tc.tile_pool(name="wstream", bufs=2) as wpool,
            tc.tile_pool(name="vpool", bufs=2) as vpool,
            tc.tile_pool(name="slab", bufs=2) as slabpool,
            tc.tile_pool(name="tmps", bufs=1) as tmppool,
            tc.tile_pool(name="acc", bufs=2) as accpool,
            tc.tile_pool(name="stage", bufs=2) as stagepool,
            tc.tile_pool(name="psum", bufs=2, space="PSUM") as psum_pool,
        ):
            dvt = singles.tile([P, OBLK], F32, tag="dvt")
            nc.sync.dma_start(out=dvt[:], in_=dv[:])

            for g in range(NGRP4):
                prow0 = g * W4M * G4TY
                xlo = max(prow0 - 1, 0)
                xhi = min(prow0 + SLAB_R - 2, H - 1)
                nrows = xhi - xlo + 1
                slab_lo = (xlo + 1) - prow0

                vts = []
                for blk in range(NBLK):
                    slab = slabpool.tile([P, SLAB_R, WP], F32, tag="slab")
                    nc.vector.memset(slab[:], 0.0)
                    nc.sync.dma_start(
                        out=slab[:, slab_lo : slab_lo + nrows, 1 : 1 + W],
                        in_=x[blk * P : (blk + 1) * P, xlo * W : (xhi + 1) * W]
                        .rearrange("p (r w) -> p r w", r=nrows),
                    )
                    vt = vpool.tile([P, W4C, G4TY, WP], F32R, tag=f"v{blk}")
                    tmp = [tmppool.tile([P, G4TY, WP], F32, tag="tmp", name="tmp")[:],
                           tmppool.tile([P, G4TY, WP], F32, tag="tmp2", name="tmp2")[:]]
                    for c in range(W4C):
                        terms = [
                            (coef, slab[:, r : r + W4M * (G4TY - 1) + 1 : W4M, :])
                            for coef, r in W4_BT[c]
                        ]
                        _emit_lincomb(nc, vt[:, c], terms, tmp)
                    vts.append(vt)

                for ocb in range(OBLK):
                    wqs = []
                    for blk in range(NBLK):
                        wq = wpool.tile([P, K, W4C, P], F32R, tag=f"wq{blk}")
                        nc.sync.dma_start(
                            out=wq[:],
                            in_=wt[blk][:, :, :, ocb * P : (ocb + 1) * P],
                        )
                        wqs.append(wq)
                    ps = psum_pool.tile([P, W4C, N4MM], F32, tag="ps")
                    for blk in range(NBLK):
                        for dx in range(K):
                            for c in range(W4C):
                                nc.tensor.matmul(
                                    ps[:, c].rearrange("p (t x) -> p t x", t=G4TY),
                                    wqs[blk][:, dx, c, :],
                                    vts[blk][:, c, :, dx : dx + W],
                                    start=(blk == 0 and dx == 0),
                                    stop=(blk == NBLK - 1 and dx == K - 1),
                                )
                    acc = accpool.tile([P, W4M, N4MM], F32, tag="acc")
                    atmp = [tmppool.tile([P, N4MM], F32, tag="atmp", name="atmp")[:],
                            tmppool.tile([P, N4MM], F32, tag="atmp2", name="atmp2")[:]]
                    for a in range(W4M):
                        terms = [(coef, ps[:, c]) for coef, c in W4_AT[a]]
                        _emit_lincomb(nc, acc[:, a], terms, atmp)
                    st = stagepool.tile([P, W4M * G4TY, W], F32, tag="st")
                    for a in range(W4M):
                        nc.scalar.activation(
                            out=st[:, a : W4M * G4TY : W4M, :],
                            in_=acc[:, a].rearrange("p (t x) -> p t x", t=G4TY),
                            func=COPY,
                            scale=dvt[:, ocb : ocb + 1],
                        )
                    nc.sync.dma_start(
                        out=out[ocb * P : (ocb + 1) * P,
                                g * W4M * G4TY * W : (g + 1) * W4M * G4TY * W],
                        in_=st[:],
                    )
    nc.finalize()
    return nc


def _host_prep(input, style, weight, ll_lora_B, ll_lora_B_inst, ll_lora_A,
               mod_weight, mod_bias, fc_lora_A, fc_lora_B, fc_lora_bias):
    f64 = np.float64
    # shared LoRA-modulated conv weight w5[oc, ic, k*k]
    Bm = np.einsum("or,brk->bok", ll_lora_B.astype(f64), ll_lora_B_inst.astype(f64))
    Bm = np.maximum(Bm, 0.0)
    w_add = np.maximum(np.einsum("ib,bok->oik", ll_lora_A.astype(f64), Bm), 0.0)
    w5 = weight.astype(f64).reshape(OUT_CH, IN_CH, K * K) + w_add

    # style modulation s[b, ic]
    w_fc = mod_weight.astype(f64) + fc_lora_B.astype(f64) @ fc_lora_A.astype(f64).T
    b_fc = mod_bias.astype(f64) + fc_lora_bias.astype(f64)
    s = style.astype(f64) @ (w_fc * MOD_SCALE).T + b_fc  # [B, in]

    # demod[b, oc] via the separable sum: sum_{i,k} (CONV_SCALE*w5*s_i)^2
    m2 = (w5 ** 2).sum(axis=2)  # [oc, ic]
    denom = (CONV_SCALE ** 2) * (s ** 2) @ m2.T + EPS  # [B, oc]
    d = CONV_SCALE / np.sqrt(denom)  # [B, oc], includes CONV_SCALE

    impl = os.environ.get("KERNEL_IMPL", "wino")
    if impl == "direct":
        # weight layout: [ic_blk, ic_in_blk, tap, oc]
        wk = np.ascontiguousarray(
            w5.transpose(1, 2, 0).reshape(NBLK, P, K * K, OUT_CH).astype(np.float32)
        )
    else:
        # Winograd row-transformed weights U[c,o,i,dx] = sum_dy G[c,dy] w5
        # laid out [ic_blk, ic, dx, c, oc]
        gmat = WINO_G if impl == "wino" else W4_G
        ncomp = WC if impl == "wino" else W4C
        u = np.einsum("cy,oiyx->coix", gmat, w5.reshape(OUT_CH, IN_CH, K, K))
        wk = np.ascontiguousarray(
            u.transpose(2, 3, 0, 1).reshape(NBLK, P, K, ncomp, OUT_CH).astype(np.float32)
        )
    return wk, s.astype(np.float32), d.astype(np.float32)


def kernel(**inputs):
    global _CACHED_NC, LAST_RESULT
    inputs = {k: np.asarray(v) for k, v in inputs.items()}
    w5T, s, d = _host_prep(**inputs)

    impl = os.environ.get("KERNEL_IMPL", "wino")
    x = np.ascontiguousarray(inputs["input"].reshape(B, IN_CH, HW), dtype=np.float32)
    in_maps = []
    for b in range(B):
        m = {
            "x": x[b],
            "dv": np.ascontiguousarray(d[b].reshape(OBLK, P).T),
        }
        if impl == "wino4":
            # style scale folded into this sample's weights on host
            m["wt"] = np.ascontiguousarray(
                w5T * s[b].reshape(NBLK, P, 1, 1, 1))
        else:
            m["wt"] = w5T
            m["sv"] = np.ascontiguousarray(s[b].reshape(NBLK, P).T)
        in_maps.append(m)

    if _CACHED_NC is None:
        if impl == "direct":
            _CACHED_NC = _build()
        elif impl == "wino":
            _CACHED_NC = _build_wino()
        else:
            _CACHED_NC = _build_wino4()

    res = run_bass_kernel_spmd(
        _CACHED_NC, in_maps, list(range(B)),
        trace=bool(os.environ.get("KERNEL_TRACE")),
    )
    LAST_RESULT = res
    out = np.stack([res.results[b]["out"] for b in range(B)])
    return out.reshape(B, OUT_CH, H, W)

